# revision 1
# baseline (speedup 1.0000x reference)
"""ALiBi mask-bias kernel for one TRN2 chip (8 NeuronCores, SPMD).

Computes out[b,h,i,j] = mask[b,h,i,j] - |slope[h] * (i - j)| for
mask shape (2, 16, 2048, 2048) f32.  q/k/v only contribute shapes in the
reference, so they are never shipped to the device.

Sharding: the 32 (b,h) pairs are split 4-per-core (data + head parallel,
no collectives needed).  Per core: read 64 MiB mask, write 64 MiB out.

Device-side math per (128, 2048) tile with row offset i0 = 128*t:
  absrel[p,f] = Abs(iota[p,f] * -1 + (i0+p))        # ScalarEngine, head-independent
  out[p,f]    = (absrel[p,f] * -slope_h) + mask[p,f] # VectorEngine scalar_tensor_tensor
The -slope_h lives in input data (per-partition scalar AP), so all 8 cores
run the identical graph.
"""

import numpy as np

import concourse.bacc as bacc
import concourse.mybir as mybir
import concourse.tile as tile
from concourse.bass_utils import run_bass_kernel_spmd

B, NH, L = 2, 16, 2048
N_CORES = 8
PPC = (B * NH) // N_CORES  # (b,h) pairs per core = 4
P = 128                    # SBUF partitions
TILES = L // P             # 16 row-tiles per (b,h) matrix


def _slopes():
    # _get_slopes(16): start = 2^(-2^(-(log2(16)-3))) = 2^-0.5; slopes[i] = start^(i+1)
    start = 2.0 ** -0.5
    return [start ** (i + 1) for i in range(NH)]


def build_graph():
    f32 = mybir.dt.float32
    nc = bacc.Bacc("TRN2", target_bir_lowering=False, debug=False, num_devices=N_CORES)

    mask_ext = nc.dram_tensor("mask", [PPC, L, L], f32, kind="ExternalInput")
    iota_ext = nc.dram_tensor("iota", [P, L], f32, kind="ExternalInput")
    rowb_ext = nc.dram_tensor("rowb", [P, TILES], f32, kind="ExternalInput")
    nslp_ext = nc.dram_tensor("nslope", [P, PPC], f32, kind="ExternalInput")
    out_ext = nc.dram_tensor("out", [PPC, L, L], f32, kind="ExternalOutput")

    with tile.TileContext(nc) as tc:
        with (
            tc.tile_pool(name="const", bufs=1) as cpool,
            tc.tile_pool(name="work", bufs=4) as pool,
        ):
            iota_t = cpool.tile([P, L], f32)
            nc.sync.dma_start(out=iota_t[:], in_=iota_ext[:, :])
            rowb_t = cpool.tile([P, TILES], f32)
            nc.sync.dma_start(out=rowb_t[:], in_=rowb_ext[:, :])
            nslp_t = cpool.tile([P, PPC], f32)
            nc.sync.dma_start(out=nslp_t[:], in_=nslp_ext[:, :])

            for t in range(TILES):
                absrel = pool.tile([P, L], f32, tag="absrel")
                nc.scalar.activation(
                    absrel[:],
                    iota_t[:],
                    mybir.ActivationFunctionType.Abs,
                    bias=rowb_t[:, t : t + 1],
                    scale=-1.0,
                )
                for h in range(PPC):
                    m = pool.tile([P, L], f32, tag="m")
                    nc.sync.dma_start(
                        out=m[:], in_=mask_ext[h, t * P : (t + 1) * P, :]
                    )
                    o = pool.tile([P, L], f32, tag="o")
                    nc.vector.scalar_tensor_tensor(
                        out=o[:],
                        in0=absrel[:],
                        scalar=nslp_t[:, h : h + 1],
                        in1=m[:],
                        op0=mybir.AluOpType.mult,
                        op1=mybir.AluOpType.add,
                    )
                    nc.sync.dma_start(
                        out=out_ext[h, t * P : (t + 1) * P, :], in_=o[:]
                    )

    nc.compile()
    return nc


_NC = None


def _get_nc():
    global _NC
    if _NC is None:
        _NC = build_graph()
    return _NC


def make_in_maps(mask):
    mask = np.ascontiguousarray(np.asarray(mask, dtype=np.float32))
    flat = mask.reshape(B * NH, L, L)
    slopes = _slopes()

    iota = np.broadcast_to(
        np.arange(L, dtype=np.float32), (P, L)
    ).copy()  # [p,f] = f
    rowb = (
        np.arange(TILES, dtype=np.float32)[None, :] * P
        + np.arange(P, dtype=np.float32)[:, None]
    ).astype(np.float32)  # [p,t] = 128*t + p

    in_maps = []
    for c in range(N_CORES):
        nsl = np.empty((P, PPC), dtype=np.float32)
        for j in range(PPC):
            nsl[:, j] = -slopes[(c * PPC + j) % NH]
        in_maps.append(
            {
                "mask": np.ascontiguousarray(flat[c * PPC : (c + 1) * PPC]),
                "iota": iota,
                "rowb": rowb,
                "nslope": nsl,
            }
        )
    return in_maps


def run(mask, trace=False, **run_kwargs):
    """Run on the 8 cores; returns (full_output, BassKernelResults)."""
    nc = _get_nc()
    res = run_bass_kernel_spmd(
        nc, make_in_maps(mask), core_ids=list(range(N_CORES)), trace=trace, **run_kwargs
    )
    out = np.concatenate(
        [np.asarray(res.results[i]["out"]) for i in range(N_CORES)], axis=0
    ).reshape(B, NH, L, L)
    return out, res


def kernel(mask, q, k, v):
    out, _ = run(mask)
    return out


# revision 6
# speedup vs baseline: 1.0076x; 1.0076x over previous
"""ALiBi mask-bias kernel for one TRN2 chip (8 NeuronCores, SPMD).

Computes out[b,h,i,j] = mask[b,h,i,j] - |slope[h] * (i - j)| for
mask shape (2, 16, 2048, 2048) f32.  q/k/v only contribute shapes in the
reference, so they are never shipped to the device.

Sharding: the 32 (b,h) pairs are split 4-per-core (data + head parallel,
no collectives needed).  Per core: read 64 MiB mask, write 64 MiB out.

Tiling: (128, 4096) tiles — each partition holds 2 consecutive mask rows,
so every DMA is a 2 MiB fully-contiguous transfer.  Device-side math:
  rel0[p,f]   = 2p + f//2048 - f%2048          # gpsimd iota, once
  absrel[p,f] = Abs(rel0[p,f] + 256*t)         # ScalarEngine, per row-tile
  out[p,f]    = (absrel * -slope_h) + mask     # VectorEngine fused op
-slope_h lives in input data (per-partition scalar AP), so all 8 cores run
the identical SPMD graph.
"""

import numpy as np

import concourse.bacc as bacc
import concourse.mybir as mybir
import concourse.tile as tile
from concourse.bass_utils import run_bass_kernel_spmd

B, NH, L = 2, 16, 2048
N_CORES = 8
PPC = (B * NH) // N_CORES  # (b,h) pairs per core = 4
P = 128                    # SBUF partitions
ROWS_PER_PART = 2
FREE = L * ROWS_PER_PART   # 4096
TILES = L // (P * ROWS_PER_PART)  # 8 row-tiles per (b,h) matrix


def _slopes():
    # _get_slopes(16): start = 2^(-2^(-(log2(16)-3))) = 2^-0.5; slopes[i] = start^(i+1)
    start = 2.0 ** -0.5
    return [start ** (i + 1) for i in range(NH)]


def build_graph():
    f32 = mybir.dt.float32
    nc = bacc.Bacc("TRN2", target_bir_lowering=False, debug=False, num_devices=N_CORES)

    mask_ext = nc.dram_tensor("mask", [PPC, L, L], f32, kind="ExternalInput")
    nslp_ext = nc.dram_tensor("nslope", [P, PPC], f32, kind="ExternalInput")
    rowb_ext = nc.dram_tensor("rowb", [P, TILES], f32, kind="ExternalInput")
    out_ext = nc.dram_tensor("out", [PPC, L, L], f32, kind="ExternalOutput")

    # (h, 2048, 2048) -> (h, t, p, f): partition p holds rows 256t+2p, 256t+2p+1
    mask_r = mask_ext.reshape([PPC, TILES, P, FREE])
    out_r = out_ext.reshape([PPC, TILES, P, FREE])

    with tile.TileContext(nc) as tc:
        with (
            tc.tile_pool(name="const", bufs=1) as cpool,
            tc.tile_pool(name="work", bufs=4) as pool,
            tc.tile_pool(name="gen", bufs=2) as gpool,
        ):
            nslp_t = cpool.tile([P, PPC], f32)
            nc.sync.dma_start(out=nslp_t[:], in_=nslp_ext[:, :])
            rowb_t = cpool.tile([P, TILES], f32)
            nc.sync.dma_start(out=rowb_t[:], in_=rowb_ext[:, :])

            # rel0[p, a*2048 + c] = 2p + a - c
            rel0 = cpool.tile([P, FREE], f32)
            nc.gpsimd.iota(
                rel0[:],
                pattern=[[1, ROWS_PER_PART], [-1, L]],
                base=0,
                channel_multiplier=ROWS_PER_PART,
                allow_small_or_imprecise_dtypes=True,
            )

            for t in range(TILES):
                absrel = gpool.tile([P, FREE], f32, tag="absrel")
                nc.scalar.activation(
                    absrel[:],
                    rel0[:],
                    mybir.ActivationFunctionType.Abs,
                    bias=rowb_t[:, t : t + 1],
                    scale=1.0,
                )
                for h in range(PPC):
                    m = pool.tile([P, FREE], f32, tag="m")
                    nc.sync.dma_start(out=m[:], in_=mask_r[h, t])
                    o = pool.tile([P, FREE], f32, tag="o")
                    nc.vector.scalar_tensor_tensor(
                        out=o[:],
                        in0=absrel[:],
                        scalar=nslp_t[:, h : h + 1],
                        in1=m[:],
                        op0=mybir.AluOpType.mult,
                        op1=mybir.AluOpType.add,
                    )
                    nc.sync.dma_start(out=out_r[h, t], in_=o[:])

    nc.compile()
    return nc


_NC = None


def _get_nc():
    global _NC
    if _NC is None:
        _NC = build_graph()
    return _NC


def make_in_maps(mask):
    mask = np.ascontiguousarray(np.asarray(mask, dtype=np.float32))
    flat = mask.reshape(B * NH, L, L)
    slopes = _slopes()

    # rowb[p, t] = 256*t (row offset of tile t; the 2p part lives in rel0)
    rowb = np.broadcast_to(
        np.arange(TILES, dtype=np.float32) * (P * ROWS_PER_PART), (P, TILES)
    ).copy()

    in_maps = []
    for c in range(N_CORES):
        nsl = np.empty((P, PPC), dtype=np.float32)
        for j in range(PPC):
            nsl[:, j] = -slopes[(c * PPC + j) % NH]
        in_maps.append(
            {
                "mask": np.ascontiguousarray(flat[c * PPC : (c + 1) * PPC]),
                "nslope": nsl,
                "rowb": rowb,
            }
        )
    return in_maps


def run(mask, trace=False, **run_kwargs):
    """Run on the 8 cores; returns (full_output, BassKernelResults)."""
    nc = _get_nc()
    res = run_bass_kernel_spmd(
        nc, make_in_maps(mask), core_ids=list(range(N_CORES)), trace=trace, **run_kwargs
    )
    out = np.concatenate(
        [np.asarray(res.results[i]["out"]) for i in range(N_CORES)], axis=0
    ).reshape(B, NH, L, L)
    return out, res


def kernel(mask, q, k, v):
    out, _ = run(mask)
    return out


# revision 8
# speedup vs baseline: 1.3023x; 1.2925x over previous
"""ALiBi mask-bias kernel for one TRN2 chip (8 NeuronCores, SPMD).

Computes out[b,h,i,j] = mask[b,h,i,j] - |slope[h] * (i - j)| for
mask shape (2, 16, 2048, 2048) f32.  q/k/v only contribute shapes in the
reference, so they are never shipped to the device.

Sharding: the 32 (b,h) pairs are split 4-per-core (data + head parallel,
no collectives needed).  Per core: read 64 MiB mask, write 64 MiB out.

Tiling: (128, 4096) tiles — each partition holds 2 consecutive mask rows,
so every DMA is a 2 MiB fully-contiguous transfer.  Device-side math:
  rel0[p,f]   = 2p + f//2048 - f%2048          # gpsimd iota, once
  absrel[p,f] = Abs(rel0[p,f] + 256*t)         # ScalarEngine, per row-tile
  out[p,f]    = (absrel * -slope_h) + mask     # VectorEngine fused op
-slope_h lives in input data (per-partition scalar AP), so all 8 cores run
the identical SPMD graph.
"""

import numpy as np

import concourse.bacc as bacc
import concourse.mybir as mybir
import concourse.tile as tile
from concourse.bass_utils import run_bass_kernel_spmd

B, NH, L = 2, 16, 2048
N_CORES = 8
PPC = (B * NH) // N_CORES  # (b,h) pairs per core = 4
P = 128                    # SBUF partitions
ROWS_PER_PART = 2
FREE = L * ROWS_PER_PART   # 4096
TILES = L // (P * ROWS_PER_PART)  # 8 row-tiles per (b,h) matrix


def _slopes():
    # _get_slopes(16): start = 2^(-2^(-(log2(16)-3))) = 2^-0.5; slopes[i] = start^(i+1)
    start = 2.0 ** -0.5
    return [start ** (i + 1) for i in range(NH)]


def build_graph():
    f32 = mybir.dt.float32
    nc = bacc.Bacc("TRN2", target_bir_lowering=False, debug=False, num_devices=N_CORES)

    mask_ext = nc.dram_tensor("mask", [PPC, L, L], f32, kind="ExternalInput")
    nslp_ext = nc.dram_tensor("nslope", [P, PPC], f32, kind="ExternalInput")
    rowb_ext = nc.dram_tensor("rowb", [P, TILES], f32, kind="ExternalInput")
    out_ext = nc.dram_tensor("out", [PPC, L, L], f32, kind="ExternalOutput")

    # (h, 2048, 2048) -> (h, t, p, f): partition p holds rows 256t+2p, 256t+2p+1
    mask_r = mask_ext.reshape([PPC, TILES, P, FREE])
    out_r = out_ext.reshape([PPC, TILES, P, FREE])

    with tile.TileContext(nc) as tc:
        with (
            tc.tile_pool(name="const", bufs=1) as cpool,
            tc.tile_pool(name="work", bufs=8) as pool,
            tc.tile_pool(name="gen", bufs=2) as gpool,
        ):
            nslp_t = cpool.tile([P, PPC], f32)
            nc.sync.dma_start(out=nslp_t[:], in_=nslp_ext[:, :])
            rowb_t = cpool.tile([P, TILES], f32)
            nc.sync.dma_start(out=rowb_t[:], in_=rowb_ext[:, :])

            # rel0[p, a*2048 + c] = 2p + a - c
            rel0 = cpool.tile([P, FREE], f32)
            nc.gpsimd.iota(
                rel0[:],
                pattern=[[1, ROWS_PER_PART], [-1, L]],
                base=0,
                channel_multiplier=ROWS_PER_PART,
                allow_small_or_imprecise_dtypes=True,
            )

            for t in range(TILES):
                absrel = gpool.tile([P, FREE], f32, tag="absrel")
                nc.scalar.activation(
                    absrel[:],
                    rel0[:],
                    mybir.ActivationFunctionType.Abs,
                    bias=rowb_t[:, t : t + 1],
                    scale=1.0,
                )
                for h in range(PPC):
                    m = pool.tile([P, FREE], f32, tag="m")
                    nc.sync.dma_start(out=m[:], in_=mask_r[h, t])
                    # in-place: m <- (absrel * -slope_h) + m
                    nc.vector.scalar_tensor_tensor(
                        out=m[:],
                        in0=absrel[:],
                        scalar=nslp_t[:, h : h + 1],
                        in1=m[:],
                        op0=mybir.AluOpType.mult,
                        op1=mybir.AluOpType.add,
                    )
                    # out-DMAs ride the Activation HWDGE ring so a pending
                    # DVE dependency can't head-of-line-block mask loads.
                    nc.scalar.dma_start(out=out_r[h, t], in_=m[:])

    nc.compile()
    return nc


_NC = None


def _get_nc():
    global _NC
    if _NC is None:
        _NC = build_graph()
    return _NC


def make_in_maps(mask):
    mask = np.ascontiguousarray(np.asarray(mask, dtype=np.float32))
    flat = mask.reshape(B * NH, L, L)
    slopes = _slopes()

    # rowb[p, t] = 256*t (row offset of tile t; the 2p part lives in rel0)
    rowb = np.broadcast_to(
        np.arange(TILES, dtype=np.float32) * (P * ROWS_PER_PART), (P, TILES)
    ).copy()

    in_maps = []
    for c in range(N_CORES):
        nsl = np.empty((P, PPC), dtype=np.float32)
        for j in range(PPC):
            nsl[:, j] = -slopes[(c * PPC + j) % NH]
        in_maps.append(
            {
                "mask": np.ascontiguousarray(flat[c * PPC : (c + 1) * PPC]),
                "nslope": nsl,
                "rowb": rowb,
            }
        )
    return in_maps


def run(mask, trace=False, **run_kwargs):
    """Run on the 8 cores; returns (full_output, BassKernelResults)."""
    nc = _get_nc()
    res = run_bass_kernel_spmd(
        nc, make_in_maps(mask), core_ids=list(range(N_CORES)), trace=trace, **run_kwargs
    )
    out = np.concatenate(
        [np.asarray(res.results[i]["out"]) for i in range(N_CORES)], axis=0
    ).reshape(B, NH, L, L)
    return out, res


def kernel(mask, q, k, v):
    out, _ = run(mask)
    return out


# revision 10
# speedup vs baseline: 1.3044x; 1.0016x over previous
"""ALiBi mask-bias kernel for one TRN2 chip (8 NeuronCores, SPMD).

Computes out[b,h,i,j] = mask[b,h,i,j] - |slope[h] * (i - j)| for
mask shape (2, 16, 2048, 2048) f32.  q/k/v only contribute shapes in the
reference, so they are never shipped to the device.

Sharding: the 32 (b,h) pairs are split 4-per-core (data + head parallel,
no collectives needed).  Per core: read 64 MiB mask, write 64 MiB out.

Tiling: (128, 4096) tiles — each partition holds 2 consecutive mask rows,
so every DMA is a 2 MiB fully-contiguous transfer.  Device-side math:
  rel0[p,f]   = 2p + f//2048 - f%2048          # gpsimd iota, once
  absrel[p,f] = Abs(rel0[p,f] + 256*t)         # ScalarEngine, per row-tile
  out[p,f]    = (absrel * -slope_h) + mask     # VectorEngine fused op
-slope_h lives in input data (per-partition scalar AP), so all 8 cores run
the identical SPMD graph.
"""

import numpy as np

import concourse.bacc as bacc
import concourse.mybir as mybir
import concourse.tile as tile
from concourse.bass_utils import run_bass_kernel_spmd

B, NH, L = 2, 16, 2048
N_CORES = 8
PPC = (B * NH) // N_CORES  # (b,h) pairs per core = 4
P = 128                    # SBUF partitions
ROWS_PER_PART = 2
FREE = L * ROWS_PER_PART   # 4096
TILES = L // (P * ROWS_PER_PART)  # 8 row-tiles per (b,h) matrix


def _slopes():
    # _get_slopes(16): start = 2^(-2^(-(log2(16)-3))) = 2^-0.5; slopes[i] = start^(i+1)
    start = 2.0 ** -0.5
    return [start ** (i + 1) for i in range(NH)]


def build_graph():
    f32 = mybir.dt.float32
    nc = bacc.Bacc("TRN2", target_bir_lowering=False, debug=False, num_devices=N_CORES)

    mask_ext = nc.dram_tensor("mask", [PPC, L, L], f32, kind="ExternalInput")
    nslp_ext = nc.dram_tensor("nslope", [P, PPC], f32, kind="ExternalInput")
    rowb_ext = nc.dram_tensor("rowb", [P, TILES], f32, kind="ExternalInput")
    out_ext = nc.dram_tensor("out", [PPC, L, L], f32, kind="ExternalOutput")

    # (h, 2048, 2048) -> (h, t, p, f): partition p holds rows 256t+2p, 256t+2p+1
    mask_r = mask_ext.reshape([PPC, TILES, P, FREE])
    out_r = out_ext.reshape([PPC, TILES, P, FREE])

    with tile.TileContext(nc) as tc:
        with (
            tc.tile_pool(name="const", bufs=1) as cpool,
            tc.tile_pool(name="work", bufs=8) as pool,
            tc.tile_pool(name="gen", bufs=2) as gpool,
        ):
            # Prefetch the first tile-group's masks before any setup work,
            # split across both HWDGE rings (the Act ring is otherwise idle
            # until the first DVE op completes).
            pre_m = []
            for h in range(PPC):
                m = pool.tile([P, FREE], f32, tag="m")
                eng = nc.sync if h % 2 == 0 else nc.scalar
                eng.dma_start(out=m[:], in_=mask_r[h, 0])
                pre_m.append(m)

            nslp_t = cpool.tile([P, PPC], f32)
            nc.sync.dma_start(out=nslp_t[:], in_=nslp_ext[:, :])
            rowb_t = cpool.tile([P, TILES], f32)
            nc.sync.dma_start(out=rowb_t[:], in_=rowb_ext[:, :])

            # rel0[p, a*2048 + c] = 2p + a - c
            rel0 = cpool.tile([P, FREE], f32)
            nc.gpsimd.iota(
                rel0[:],
                pattern=[[1, ROWS_PER_PART], [-1, L]],
                base=0,
                channel_multiplier=ROWS_PER_PART,
                allow_small_or_imprecise_dtypes=True,
            )

            for t in range(TILES):
                absrel = gpool.tile([P, FREE], f32, tag="absrel")
                nc.scalar.activation(
                    absrel[:],
                    rel0[:],
                    mybir.ActivationFunctionType.Abs,
                    bias=rowb_t[:, t : t + 1],
                    scale=1.0,
                )
                for h in range(PPC):
                    if t == 0:
                        m = pre_m[h]
                    else:
                        m = pool.tile([P, FREE], f32, tag="m")
                        nc.sync.dma_start(out=m[:], in_=mask_r[h, t])
                    # in-place: m <- (absrel * -slope_h) + m
                    nc.vector.scalar_tensor_tensor(
                        out=m[:],
                        in0=absrel[:],
                        scalar=nslp_t[:, h : h + 1],
                        in1=m[:],
                        op0=mybir.AluOpType.mult,
                        op1=mybir.AluOpType.add,
                    )
                    # out-DMAs ride the Activation HWDGE ring so a pending
                    # DVE dependency can't head-of-line-block mask loads.
                    nc.scalar.dma_start(out=out_r[h, t], in_=m[:])

    nc.compile()
    return nc


_NC = None


def _get_nc():
    global _NC
    if _NC is None:
        _NC = build_graph()
    return _NC


def make_in_maps(mask):
    mask = np.ascontiguousarray(np.asarray(mask, dtype=np.float32))
    flat = mask.reshape(B * NH, L, L)
    slopes = _slopes()

    # rowb[p, t] = 256*t (row offset of tile t; the 2p part lives in rel0)
    rowb = np.broadcast_to(
        np.arange(TILES, dtype=np.float32) * (P * ROWS_PER_PART), (P, TILES)
    ).copy()

    in_maps = []
    for c in range(N_CORES):
        nsl = np.empty((P, PPC), dtype=np.float32)
        for j in range(PPC):
            nsl[:, j] = -slopes[(c * PPC + j) % NH]
        in_maps.append(
            {
                "mask": np.ascontiguousarray(flat[c * PPC : (c + 1) * PPC]),
                "nslope": nsl,
                "rowb": rowb,
            }
        )
    return in_maps


def run(mask, trace=False, **run_kwargs):
    """Run on the 8 cores; returns (full_output, BassKernelResults)."""
    nc = _get_nc()
    res = run_bass_kernel_spmd(
        nc, make_in_maps(mask), core_ids=list(range(N_CORES)), trace=trace, **run_kwargs
    )
    out = np.concatenate(
        [np.asarray(res.results[i]["out"]) for i in range(N_CORES)], axis=0
    ).reshape(B, NH, L, L)
    return out, res


def kernel(mask, q, k, v):
    out, _ = run(mask)
    return out
